# revision 20
# baseline (speedup 1.0000x reference)
"""Multi-head causal attention (B=2, S=2048, D=1024, H=16) on 8 TRN2 cores.

Sharding (Megatron-style): core c handles batch b = c//4, head-group
g = c%4 (4 heads, d' slice of 256). Each core computes its partial
out = ctx_g @ Wo[gslice] (no bias) in fp16; host sums the 4 partials
per batch in fp32 and adds the bias.

Device kernel dataflow (fp16 matmul operands, fp32 PSUM accumulation):
  qT/kT [d', S] and v via PE projections ->
  scores sT[k, q] per (head-pair, k-tile, q-block) (row-packed K=64
  matmul pairs) -> exp on ACT (psum->sbuf, fused 1/sqrt(hd) scale) ->
  causal triangle masking via GPSIMD affine_select in-place; fully
  masked regions are simply skipped by partial-width matmuls ->
  ctxT + softmax denominators accumulated on PE (ones columns
  interleaved in the v operand) -> 1/denom via DVE reciprocal ->
  normalize on DVE (shifted-in1 mul) -> out projection on PE.

Schedule: single j-loop interleaving both head pairs per q-block so the
ACT exp load is spread across the whole kernel instead of bunching in a
second phase (which measured ACT-bound + power-throttled). Out-proj
chunks ride at the end of each j so the output DMA streams throughout.
"""

import numpy as np

import concourse.bass as bass
import concourse.mybir as mybir
import concourse.tile as tile
from concourse import bacc, bass_utils
from concourse.hw_specs import get_activation_tables

F32 = mybir.dt.float32
F16 = mybir.dt.float16
EXP = mybir.ActivationFunctionType.Exp
LN = mybir.ActivationFunctionType.Ln

B, S, D, H, HD = 2, 2048, 1024, 16, 64
NHL = 4          # local heads per core
DC = NHL * HD    # 256 local d'
NDT = D // 128   # 8 contraction tiles for projections
KT = 128         # k tile
NKT = S // KT    # 16
QB = 512         # q block
NQB = S // QB    # 4
SCALE = 1.0 / np.sqrt(HD)

# va free-layout per k-tile m: [v0 | J | v1 | v2 | J | v3], J = ones(64)
# (matmul weight APs allow only one free dim, so the ones blocks are
# interleaved to make every head a contiguous 128-col slice).
# Head h reads 128 cols at VA_OFF[h]; even heads are [v|J] (ctx psum rows
# 0:64, denom 64:128), odd heads [J|v] (denom 0:64, ctx 64:128).
VA_W = 384
VA_OFF = [0, 64, 192, 256]


def _pin_act_table(arch):
    """Steer Bacc's activation-table chooser to the one set containing both
    exp and ln so ACT never thrashes ACT_TABLE_LOADs (1.28us each)."""
    tabs = get_activation_tables(arch)
    keep = "natural_log_exp_and_others"
    if keep not in tabs:
        return
    for name, funcs in tabs.items():
        if name != keep:
            funcs.discard(EXP)
            funcs.discard(LN)


def build_nc():
    nc = bacc.Bacc("TRN2", target_bir_lowering=False, debug=False)
    xT = nc.dram_tensor("xT", [128, NDT, S], F16, kind="ExternalInput")
    wq = nc.dram_tensor("wq", [128, 2, NDT, 128], F16, kind="ExternalInput")
    wk = nc.dram_tensor("wk", [128, 2, NDT, 128], F16, kind="ExternalInput")
    wv = nc.dram_tensor("wv", [128, NDT, DC], F16, kind="ExternalInput")
    wo = nc.dram_tensor("wo", [128, 2, D], F16, kind="ExternalInput")
    out = nc.dram_tensor("out_p", [S, D], F16, kind="ExternalOutput")

    with tile.TileContext(nc) as tc:
        with (
            tc.tile_pool(name="xp", bufs=1) as xp,
            tc.tile_pool(name="wp", bufs=1) as wp,
            tc.tile_pool(name="qk", bufs=1) as qkp,
            tc.tile_pool(name="vap", bufs=1) as vap,
            tc.tile_pool(name="cnp", bufs=1) as cnp,
            tc.tile_pool(name="et", bufs=8) as etp,
            tc.tile_pool(name="rcp", bufs=4) as rcp,
            tc.tile_pool(name="ob", bufs=4) as obp,
            tc.tile_pool(name="pp", bufs=2, space="PSUM") as pp,
            tc.tile_pool(name="sp", bufs=2, space="PSUM") as sp,
            tc.tile_pool(name="cp", bufs=1, space="PSUM") as cp,
        ):
            # ---- loads: pair-0 weight halves first so the j=0 q/k chains
            # can start as soon as x block 0 lands; x column-blocked on the
            # sync queue; wv/wo ride the vector engine's queue.
            wq_sb = wp.tile([128, 2, NDT, 128], F16, tag="wq")
            wk_sb = wp.tile([128, 2, NDT, 128], F16, tag="wk")
            wv_sb = wp.tile([128, NDT, DC], F16, tag="wv")
            wo_sb = wp.tile([128, 2, D], F16, tag="wo")
            x_sb = xp.tile([128, NDT, S], F16)
            # pair-0 q/k weights lead the scalar HWDGE queue (land ~9-11us);
            # x block 0 is split by t-halves on the sync queue so the first
            # projection chain can start accumulating after 512KiB; pair-1
            # weights ride the (slow-starting) gpsimd SWDGE queue — they
            # aren't needed until ~25us.
            # q1 (sync) carries ONLY x block 0 — a queue interleaves its
            # pending DMAs, so anything else on it delays the critical
            # first block.  Everything else streams on the scalar queue in
            # need order; pair-1 q/k weights ride the gpsimd SWDGE queue.
            nc.sync.dma_start(x_sb[:, 0:4, 0:QB], xT.ap()[:, 0:4, 0:QB])
            nc.sync.dma_start(x_sb[:, 4:8, 0:QB], xT.ap()[:, 4:8, 0:QB])
            nc.scalar.dma_start(wq_sb[:, 0], wq.ap()[:, 0])
            nc.scalar.dma_start(wk_sb[:, 0], wk.ap()[:, 0])
            nc.scalar.dma_start(wv_sb[:], wv.ap())
            nc.scalar.dma_start(x_sb[:, :, QB:2 * QB], xT.ap()[:, :, QB:2 * QB])
            nc.scalar.dma_start(wo_sb[:], wo.ap())
            for j in range(2, NQB):
                nc.scalar.dma_start(x_sb[:, :, QB * j:QB * (j + 1)],
                                    xT.ap()[:, :, QB * j:QB * (j + 1)])
            xt_sb = [x_sb[:, t, :] for t in range(NDT)]

            # ---- constants; warmup matmuls ramp the HAM clock gate while
            # the input DMAs stream. wu needs only a fast gpsimd memset.
            wu = wp.tile([128, 128], F16, tag="wu")
            nc.gpsimd.memset(wu[:], 1.0)
            nc.gpsimd.dma_start(wq_sb[:, 1], wq.ap()[:, 1])
            nc.gpsimd.dma_start(wk_sb[:, 1], wk.ap()[:, 1])
            wups = pp.tile([128, 128], F32, tag="pp", name="wups")
            for _ in range(48):
                nc.tensor.matmul(wups[:], wu[:], wu[:], start=True, stop=True)
            tri = wp.tile([128, 128], F16, tag="tri")
            nc.gpsimd.memset(tri[:], 1.0)
            # tri[k, q] = 1 if q >= k else 0
            nc.gpsimd.affine_select(
                out=tri[:], in_=tri[:], compare_op=mybir.AluOpType.is_ge,
                fill=0.0, base=0, pattern=[[1, 128]], channel_multiplier=-1)
            # [128, NKT, 2, 192] view of the [v0|J|v1|v2|J|v3] layout: each
            # head's 128-col window lives inside one 192 block, and the
            # (v0,v2)/(v1,v3) eviction pairs and both J strips become single
            # strided APs.
            va = vap.tile([128, NKT, 2, VA_W // 2], F16)
            # only the ones-strips J; v strips are written by proj_v
            nc.gpsimd.memset(va[:, :, :, 64:128], 1.0)
            warmup = wp.tile([1, 8], F32, tag="wuact")
            nc.vector.memset(warmup[:], 1.0)
            nc.scalar.activation(warmup[:], warmup[:], EXP, scale=0.001)

            # ---- projections ----
            qT = [qkp.tile([128, S], F16, tag=f"q{hp}", name=f"q{hp}")
                  for hp in range(2)]
            kTt = [qkp.tile([128, S], F16, tag=f"k{hp}", name=f"k{hp}")
                   for hp in range(2)]
            ctxn = [cnp.tile([128, S], F16, tag=f"c{t}", name=f"c{t}")
                    for t in range(2)]

            def proj_chain(hp, dst, w_sb, j):
                ps = pp.tile([128, QB], F32, tag="pp")
                for t in range(NDT):
                    nc.tensor.matmul(
                        ps[:], w_sb[:, hp, t, :],
                        xt_sb[t][:, QB * j:QB * (j + 1)],
                        start=(t == 0), stop=(t == NDT - 1))
                nc.vector.tensor_copy(dst[:, QB * j:QB * (j + 1)], ps[:])

            def proj_v(lo, hi):
                for m in range(lo, hi):
                    ps = pp.tile([128, 2, DC // 2], F32, tag="pp")
                    for t in range(NDT):
                        nc.tensor.matmul(
                            ps[:, :, :], xt_sb[t][:, 128 * m:128 * (m + 1)],
                            wv_sb[:, t, :], start=(t == 0), stop=(t == NDT - 1))
                    # 2 strided copies: (v0,v2) -> block starts, (v1,v3) ->
                    # block offsets 128.
                    nc.vector.tensor_copy(va[:, m, :, 0:64], ps[:, :, 0:64])
                    nc.vector.tensor_copy(va[:, m, :, 128:192],
                                          ps[:, :, 64:128])

            def attention_j(hp, j):
                cpb = cp.tile([128, 2, QB], F32, tag="cp")  # heads 2hp, 2hp+1
                cpe, cpo = cpb[:, 0], cpb[:, 1]
                nkt_j = 4 * j + 4
                for i in range(nkt_j):
                    c0 = max(0, 128 * (i - 4 * j))
                    spt = sp.tile([128, 2, QB], F32, tag="sp")
                    for lh in range(2):
                        nc.tensor.matmul(
                            spt[:, lh, c0:QB],
                            kTt[hp][64 * lh:64 * (lh + 1), 128 * i:128 * (i + 1)],
                            qT[hp][64 * lh:64 * (lh + 1), QB * j + c0:QB * (j + 1)],
                            start=True, stop=True)
                    et = etp.tile([128, 2, QB], F16, tag="et")
                    nc.scalar.activation(et[:, :, c0:QB], spt[:, :, c0:QB],
                                         EXP, scale=float(SCALE))
                    if i >= 4 * j:  # diagonal: triangle mask in place
                        sl = et[:, :, c0:c0 + 128]
                        nc.gpsimd.affine_select(
                            out=sl, in_=sl,
                            compare_op=mybir.AluOpType.is_ge, fill=0.0,
                            base=0, pattern=[[0, 2], [1, 128]],
                            channel_multiplier=-1)
                    for lh, cpt in ((0, cpe), (1, cpo)):
                        off = VA_OFF[2 * hp + lh]
                        blk, o = off // 192, off % 192
                        nc.tensor.matmul(
                            cpt[:, c0:QB],
                            va[:, i, blk, o:o + 128],
                            et[:, lh, c0:QB],
                            start=(i == 0), stop=(i == nkt_j - 1))
                # normalize; even heads [v|J]: ctx rows 0:64, denom 64:128;
                # odd heads [J|v]: denom 0:64, ctx 64:128.  1/denom =
                # exp(-ln(denom)) on ACT over the whole [128, 2, QB] block
                # (one ln + one exp for both heads; ctx-row lanes produce
                # junk that is never read; DVE reciprocal measured 3.3us
                # per tile and gpsimd divide doesn't compile), then
                # shifted-in1 DVE muls (verified exact on HW).
                # high_priority: the ln/exp must beat the NEXT attention's
                # queued exp tiles on ACT — the following attention's ctx
                # accumulation WARs on this block's psum, so the normalize
                # chain is latency-critical at every block boundary.
                t1 = rcp.tile([128, 2, QB], F32, tag="t1")
                rc = rcp.tile([128, 2, QB], F32, tag="rc")
                with tc.high_priority(offset=200):
                    nc.scalar.activation(t1[:], cpb[:], LN)
                    nc.scalar.activation(rc[:], t1[:], EXP, scale=-1.0)
                    for lh, cpt in ((0, cpe), (1, cpo)):
                        cr = slice(64 * lh, 64 * lh + 64)        # ctx rows
                        dr = slice(64 - 64 * lh, 128 - 64 * lh)  # denom rows
                        nc.vector.tensor_mul(
                            ctxn[hp][cr, QB * j:QB * (j + 1)], cpt[cr, :],
                            rc[dr, lh, :])

            def out_chunk(m, split_engines=False):
                ot = obp.tile([128, D], F16, tag="ob")
                for o in range(2):
                    ps = pp.tile([128, QB], F32, tag="pp")
                    for t in range(2):
                        nc.tensor.matmul(
                            ps[:], ctxn[t][:, 128 * m:128 * (m + 1)],
                            wo_sb[:, t, QB * o:QB * (o + 1)],
                            start=(t == 0), stop=(t == 1))
                    # evictions ride the DVE; on the last block the o=1
                    # half goes via ACT Copy so the two halves drain in
                    # parallel and the tail shortens.
                    osl = slice(QB * o, QB * (o + 1))
                    if split_engines and o == 1:
                        nc.scalar.activation(
                            ot[:, osl], ps[:],
                            mybir.ActivationFunctionType.Copy)
                    else:
                        nc.vector.tensor_copy(ot[:, osl], ps[:])
                    nc.sync.dma_start(
                        out.ap()[128 * m:128 * (m + 1), osl], ot[:, osl])

            # Interleaved schedule: per q-block j run pair-0 attention, then
            # pair-1 projections ride behind pair-0's exp, then pair-1
            # attention; the NEXT block's pair-0 projections fill the PE
            # while pair-1's normalize drains, then the out-proj chunks for
            # block j (which need both pairs' ctxn) close it out.
            proj_chain(0, qT[0], wq_sb, 0)
            proj_chain(0, kTt[0], wk_sb, 0)
            proj_v(0, 4)
            for j in range(NQB):
                attention_j(0, j)
                proj_chain(1, qT[1], wq_sb, j)
                proj_chain(1, kTt[1], wk_sb, j)
                attention_j(1, j)
                if j + 1 < NQB:
                    proj_chain(0, qT[0], wq_sb, j + 1)
                    proj_chain(0, kTt[0], wk_sb, j + 1)
                    proj_v(4 * (j + 1), 4 * (j + 2))
                for m in range(4 * j, 4 * j + 4):
                    out_chunk(m, split_engines=(j == NQB - 1))
    _pin_act_table(nc.m.arch)
    nc.compile()
    return nc


_NC = None


def _get_nc():
    global _NC
    if _NC is None:
        _NC = build_nc()
    return _NC


def make_in_maps(x, Wq, Wk, Wv, Wo):
    x = np.asarray(x, np.float32)
    Wq, Wk, Wv, Wo = (np.asarray(w, np.float32) for w in (Wq, Wk, Wv, Wo))
    in_maps = []
    for c in range(8):
        b, g = c // 4, c % 4
        sl = slice(DC * g, DC * (g + 1))
        # wq/wk host layout [128, 2, NDT, 128]: [p, hp, t, c] =
        # W[t*128+p, 256g + 128hp + c] so each pair half is one contiguous
        # DMA and chains slice [:, hp, t, :].
        wq_l = (Wq[:, sl].astype(np.float16).reshape(NDT, 128, 2, 128)
                .transpose(1, 2, 0, 3))
        wk_l = (Wk[:, sl].astype(np.float16).reshape(NDT, 128, 2, 128)
                .transpose(1, 2, 0, 3))
        in_maps.append({
            "xT": np.ascontiguousarray(
                x[b].T.astype(np.float16).reshape(NDT, 128, S)
                .transpose(1, 0, 2)),
            "wq": np.ascontiguousarray(wq_l),
            "wk": np.ascontiguousarray(wk_l),
            "wv": np.ascontiguousarray(
                Wv[:, sl].astype(np.float16).reshape(NDT, 128, DC)
                .transpose(1, 0, 2)),
            "wo": np.ascontiguousarray(
                Wo[sl, :].astype(np.float16).reshape(2, 128, D)
                .transpose(1, 0, 2)),
        })
    return in_maps


def kernel(x, Wq, Wk, Wv, Wo, bo, _trace=False, _trace_cores=None):
    nc = _get_nc()
    in_maps = make_in_maps(x, Wq, Wk, Wv, Wo)
    res = bass_utils.run_bass_kernel_spmd(
        nc, in_maps, core_ids=list(range(8)), trace=_trace,
        trace_cores=_trace_cores)
    bo = np.asarray(bo, np.float32)
    out = np.empty((B, S, D), np.float32)
    for b in range(B):
        acc = res.results[4 * b]["out_p"].astype(np.float32)
        for g in range(1, 4):
            acc += res.results[4 * b + g]["out_p"].astype(np.float32)
        out[b] = acc + bo
    kernel.last_results = res
    return out


# revision 24
# speedup vs baseline: 1.0029x; 1.0029x over previous
"""Multi-head causal attention (B=2, S=2048, D=1024, H=16) on 8 TRN2 cores.

Sharding (Megatron-style): core c handles batch b = c//4, head-group
g = c%4 (4 heads, d' slice of 256). Each core computes its partial
out = ctx_g @ Wo[gslice] (no bias) in fp16; host sums the 4 partials
per batch in fp32 and adds the bias.

Device kernel dataflow (fp16 matmul operands, fp32 PSUM accumulation):
  qT/kT [d', S] and v via PE projections ->
  scores sT[k, q] per (head-pair, k-tile, q-block) (row-packed K=64
  matmul pairs) -> exp on ACT (psum->sbuf, fused 1/sqrt(hd) scale) ->
  causal triangle masking via GPSIMD affine_select in-place; fully
  masked regions are simply skipped by partial-width matmuls ->
  ctxT + softmax denominators accumulated on PE (ones columns
  interleaved in the v operand) -> 1/denom via DVE reciprocal ->
  normalize on DVE (shifted-in1 mul) -> out projection on PE.

Schedule: single j-loop interleaving both head pairs per q-block so the
ACT exp load is spread across the whole kernel instead of bunching in a
second phase (which measured ACT-bound + power-throttled). Out-proj
chunks ride at the end of each j so the output DMA streams throughout.
"""

import numpy as np

import concourse.bass as bass
import concourse.mybir as mybir
import concourse.tile as tile
from concourse import bacc, bass_utils
from concourse.hw_specs import get_activation_tables

F32 = mybir.dt.float32
F16 = mybir.dt.float16
EXP = mybir.ActivationFunctionType.Exp
LN = mybir.ActivationFunctionType.Ln

B, S, D, H, HD = 2, 2048, 1024, 16, 64
NHL = 4          # local heads per core
DC = NHL * HD    # 256 local d'
NDT = D // 128   # 8 contraction tiles for projections
KT = 128         # k tile
NKT = S // KT    # 16
QB = 512         # q block
NQB = S // QB    # 4
SCALE = 1.0 / np.sqrt(HD)

# va free-layout per k-tile m: [v0 | J | v1 | v2 | J | v3], J = ones(64)
# (matmul weight APs allow only one free dim, so the ones blocks are
# interleaved to make every head a contiguous 128-col slice).
# Head h reads 128 cols at VA_OFF[h]; even heads are [v|J] (ctx psum rows
# 0:64, denom 64:128), odd heads [J|v] (denom 0:64, ctx 64:128).
VA_W = 384
VA_OFF = [0, 64, 192, 256]


def _pin_act_table(arch):
    """Steer Bacc's activation-table chooser to the one set containing both
    exp and ln so ACT never thrashes ACT_TABLE_LOADs (1.28us each)."""
    tabs = get_activation_tables(arch)
    keep = "natural_log_exp_and_others"
    if keep not in tabs:
        return
    for name, funcs in tabs.items():
        if name != keep:
            funcs.discard(EXP)
            funcs.discard(LN)


def build_nc():
    nc = bacc.Bacc("TRN2", target_bir_lowering=False, debug=False)
    xT = nc.dram_tensor("xT", [128, NDT, S], F16, kind="ExternalInput")
    wq = nc.dram_tensor("wq", [128, 2, NDT, 128], F16, kind="ExternalInput")
    wk = nc.dram_tensor("wk", [128, 2, NDT, 128], F16, kind="ExternalInput")
    wv = nc.dram_tensor("wv", [128, NDT, DC], F16, kind="ExternalInput")
    wo = nc.dram_tensor("wo", [128, 2, D], F16, kind="ExternalInput")
    out = nc.dram_tensor("out_p", [S, D], F16, kind="ExternalOutput")

    with tile.TileContext(nc) as tc:
        with (
            tc.tile_pool(name="xp", bufs=1) as xp,
            tc.tile_pool(name="wp", bufs=1) as wp,
            tc.tile_pool(name="qk", bufs=1) as qkp,
            tc.tile_pool(name="vap", bufs=1) as vap,
            tc.tile_pool(name="cnp", bufs=1) as cnp,
            tc.tile_pool(name="et", bufs=8) as etp,
            tc.tile_pool(name="rcp", bufs=4) as rcp,
            tc.tile_pool(name="ob", bufs=4) as obp,
            tc.tile_pool(name="pp", bufs=2, space="PSUM") as pp,
            tc.tile_pool(name="sp", bufs=2, space="PSUM") as sp,
            tc.tile_pool(name="cp", bufs=1, space="PSUM") as cp,
        ):
            # ---- loads: pair-0 weight halves first so the j=0 q/k chains
            # can start as soon as x block 0 lands; x column-blocked on the
            # sync queue; wv/wo ride the vector engine's queue.
            wq_sb = wp.tile([128, 2, NDT, 128], F16, tag="wq")
            wk_sb = wp.tile([128, 2, NDT, 128], F16, tag="wk")
            wv_sb = wp.tile([128, NDT, DC], F16, tag="wv")
            wo_sb = wp.tile([128, 2, D], F16, tag="wo")
            x_sb = xp.tile([128, NDT, S], F16)
            # pair-0 q/k weights lead the scalar HWDGE queue (land ~9-11us);
            # x block 0 is split by t-halves on the sync queue so the first
            # projection chain can start accumulating after 512KiB; pair-1
            # weights ride the (slow-starting) gpsimd SWDGE queue — they
            # aren't needed until ~25us.
            # q1 (sync) carries ONLY x block 0 — a queue interleaves its
            # pending DMAs, so anything else on it delays the critical
            # first block.  Everything else streams on the scalar queue in
            # need order; pair-1 q/k weights ride the gpsimd SWDGE queue.
            nc.sync.dma_start(x_sb[:, 0:4, 0:QB], xT.ap()[:, 0:4, 0:QB])
            nc.sync.dma_start(x_sb[:, 4:8, 0:QB], xT.ap()[:, 4:8, 0:QB])
            nc.scalar.dma_start(wq_sb[:, 0], wq.ap()[:, 0])
            nc.scalar.dma_start(wk_sb[:, 0], wk.ap()[:, 0])
            nc.scalar.dma_start(wv_sb[:], wv.ap())
            if NQB > 1:
                nc.scalar.dma_start(x_sb[:, :, QB:2 * QB],
                                    xT.ap()[:, :, QB:2 * QB])
            nc.scalar.dma_start(wo_sb[:], wo.ap())
            for j in range(2, NQB):
                nc.scalar.dma_start(x_sb[:, :, QB * j:QB * (j + 1)],
                                    xT.ap()[:, :, QB * j:QB * (j + 1)])
            xt_sb = [x_sb[:, t, :] for t in range(NDT)]

            # ---- constants; warmup matmuls ramp the HAM clock gate while
            # the input DMAs stream. wu needs only a fast gpsimd memset.
            wu = wp.tile([128, 128], F16, tag="wu")
            nc.gpsimd.memset(wu[:], 1.0)
            nc.gpsimd.dma_start(wq_sb[:, 1], wq.ap()[:, 1])
            nc.gpsimd.dma_start(wk_sb[:, 1], wk.ap()[:, 1])
            wups = pp.tile([128, 128], F32, tag="pp", name="wups")
            for _ in range(48):
                nc.tensor.matmul(wups[:], wu[:], wu[:], start=True, stop=True)
            tri = wp.tile([128, 128], F16, tag="tri")
            nc.gpsimd.memset(tri[:], 1.0)
            # tri[k, q] = 1 if q >= k else 0
            nc.gpsimd.affine_select(
                out=tri[:], in_=tri[:], compare_op=mybir.AluOpType.is_ge,
                fill=0.0, base=0, pattern=[[1, 128]], channel_multiplier=-1)
            # [128, NKT, 2, 192] view of the [v0|J|v1|v2|J|v3] layout: each
            # head's 128-col window lives inside one 192 block, and the
            # (v0,v2)/(v1,v3) eviction pairs and both J strips become single
            # strided APs.
            va = vap.tile([128, NKT, 2, VA_W // 2], F16)
            # only the ones-strips J; v strips are written by proj_v
            nc.gpsimd.memset(va[:, :, :, 64:128], 1.0)
            warmup = wp.tile([1, 8], F32, tag="wuact")
            nc.vector.memset(warmup[:], 1.0)
            nc.scalar.activation(warmup[:], warmup[:], EXP, scale=0.001)

            # ---- projections ----
            qT = [qkp.tile([128, S], F16, tag=f"q{hp}", name=f"q{hp}")
                  for hp in range(2)]
            kTt = [qkp.tile([128, S], F16, tag=f"k{hp}", name=f"k{hp}")
                   for hp in range(2)]
            ctxn = [cnp.tile([128, S], F16, tag=f"c{t}", name=f"c{t}")
                    for t in range(2)]

            def proj_chain(hp, dst, w_sb, j):
                ps = pp.tile([128, QB], F32, tag="pp")
                for t in range(NDT):
                    nc.tensor.matmul(
                        ps[:], w_sb[:, hp, t, :],
                        xt_sb[t][:, QB * j:QB * (j + 1)],
                        start=(t == 0), stop=(t == NDT - 1))
                nc.vector.tensor_copy(dst[:, QB * j:QB * (j + 1)], ps[:])

            def proj_v(lo, hi):
                for m in range(lo, hi):
                    ps = pp.tile([128, 2, DC // 2], F32, tag="pp")
                    for t in range(NDT):
                        nc.tensor.matmul(
                            ps[:, :, :], xt_sb[t][:, 128 * m:128 * (m + 1)],
                            wv_sb[:, t, :], start=(t == 0), stop=(t == NDT - 1))
                    # 2 strided copies: (v0,v2) -> block starts, (v1,v3) ->
                    # block offsets 128.
                    nc.vector.tensor_copy(va[:, m, :, 0:64], ps[:, :, 0:64])
                    nc.vector.tensor_copy(va[:, m, :, 128:192],
                                          ps[:, :, 64:128])

            def attention_j(hp, j, fillers=()):
                cpb = cp.tile([128, 2, QB], F32, tag="cp")  # heads 2hp, 2hp+1
                cpe, cpo = cpb[:, 0], cpb[:, 1]
                nkt_j = 4 * j + 4
                # filler PE units (proj chains / out chunks) are emitted
                # between the scores and ctx matmuls of evenly spread
                # i-tiles: within attention the ACT exp (~1.15us/tile)
                # outruns the PE (~0.86us/tile), so without filler the PE
                # starves; the first unit also covers the ctx-psum WAR on
                # the previous attention's normalize.
                nf = len(fillers)
                due = sorted(k * nkt_j // nf for k in range(nf)) if nf else []
                fi = 0
                for i in range(nkt_j):
                    c0 = max(0, 128 * (i - 4 * j))
                    spt = sp.tile([128, 2, QB], F32, tag="sp")
                    for lh in range(2):
                        nc.tensor.matmul(
                            spt[:, lh, c0:QB],
                            kTt[hp][64 * lh:64 * (lh + 1), 128 * i:128 * (i + 1)],
                            qT[hp][64 * lh:64 * (lh + 1), QB * j + c0:QB * (j + 1)],
                            start=True, stop=True)
                    while fi < nf and due[fi] == i:
                        fillers[fi]()
                        fi += 1
                    et = etp.tile([128, 2, QB], F16, tag="et")
                    nc.scalar.activation(et[:, :, c0:QB], spt[:, :, c0:QB],
                                         EXP, scale=float(SCALE))
                    if i >= 4 * j:  # diagonal: triangle mask in place
                        sl = et[:, :, c0:c0 + 128]
                        nc.gpsimd.affine_select(
                            out=sl, in_=sl,
                            compare_op=mybir.AluOpType.is_ge, fill=0.0,
                            base=0, pattern=[[0, 2], [1, 128]],
                            channel_multiplier=-1)
                    for lh, cpt in ((0, cpe), (1, cpo)):
                        off = VA_OFF[2 * hp + lh]
                        blk, o = off // 192, off % 192
                        nc.tensor.matmul(
                            cpt[:, c0:QB],
                            va[:, i, blk, o:o + 128],
                            et[:, lh, c0:QB],
                            start=(i == 0), stop=(i == nkt_j - 1))
                # normalize; even heads [v|J]: ctx rows 0:64, denom 64:128;
                # odd heads [J|v]: denom 0:64, ctx 64:128.  1/denom =
                # exp(-ln(denom)) on ACT over the whole [128, 2, QB] block
                # (one ln + one exp for both heads; ctx-row lanes produce
                # junk that is never read; DVE reciprocal measured 3.3us
                # per tile and gpsimd divide doesn't compile), then
                # shifted-in1 DVE muls (verified exact on HW).
                t1 = rcp.tile([128, 2, QB], F32, tag="t1")
                rc = rcp.tile([128, 2, QB], F32, tag="rc")
                nc.scalar.activation(t1[:], cpb[:], LN)
                nc.scalar.activation(rc[:], t1[:], EXP, scale=-1.0)
                for lh, cpt in ((0, cpe), (1, cpo)):
                    cr = slice(64 * lh, 64 * lh + 64)        # ctx rows
                    dr = slice(64 - 64 * lh, 128 - 64 * lh)  # denom rows
                    nc.vector.tensor_mul(
                        ctxn[hp][cr, QB * j:QB * (j + 1)], cpt[cr, :],
                        rc[dr, lh, :])

            def out_chunk(m, split_engines=False):
                ot = obp.tile([128, D], F16, tag="ob")
                for o in range(2):
                    ps = pp.tile([128, QB], F32, tag="pp")
                    for t in range(2):
                        nc.tensor.matmul(
                            ps[:], ctxn[t][:, 128 * m:128 * (m + 1)],
                            wo_sb[:, t, QB * o:QB * (o + 1)],
                            start=(t == 0), stop=(t == 1))
                    # evictions ride the DVE; on the last block the o=1
                    # half goes via ACT Copy so the two halves drain in
                    # parallel and the tail shortens.
                    osl = slice(QB * o, QB * (o + 1))
                    if split_engines and o == 1:
                        nc.scalar.activation(
                            ot[:, osl], ps[:],
                            mybir.ActivationFunctionType.Copy)
                    else:
                        nc.vector.tensor_copy(ot[:, osl], ps[:])
                    nc.sync.dma_start(
                        out.ap()[128 * m:128 * (m + 1), osl], ot[:, osl])

            # Interleaved schedule: per q-block j, pair-0 attention carries
            # pair-1's block-j projections plus block j-1's out-proj
            # chunks as fillers; pair-1 attention carries the NEXT block's
            # pair-0 projections and v tiles.  The final block's out
            # chunks drain with split-engine evictions.
            proj_chain(0, qT[0], wq_sb, 0)
            proj_chain(0, kTt[0], wk_sb, 0)
            proj_v(0, 4)
            for j in range(NQB):
                f0 = [lambda j=j: proj_chain(1, qT[1], wq_sb, j),
                      lambda j=j: proj_chain(1, kTt[1], wk_sb, j)]
                if j >= 1:
                    f0 += [lambda m=m: out_chunk(m)
                           for m in range(4 * (j - 1), 4 * j)]
                attention_j(0, j, f0)
                f1 = []
                if j + 1 < NQB:
                    f1 = [lambda j=j: proj_chain(0, qT[0], wq_sb, j + 1),
                          lambda j=j: proj_chain(0, kTt[0], wk_sb, j + 1)]
                    f1 += [lambda m=m: proj_v(m, m + 1)
                           for m in range(4 * (j + 1), 4 * (j + 2))]
                attention_j(1, j, f1)
            for m in range(4 * (NQB - 1), 4 * NQB):
                out_chunk(m, split_engines=True)
    _pin_act_table(nc.m.arch)
    nc.compile()
    return nc


_NC = None


def _get_nc():
    global _NC
    if _NC is None:
        _NC = build_nc()
    return _NC


def make_in_maps(x, Wq, Wk, Wv, Wo):
    x = np.asarray(x, np.float32)
    Wq, Wk, Wv, Wo = (np.asarray(w, np.float32) for w in (Wq, Wk, Wv, Wo))
    in_maps = []
    for c in range(8):
        b, g = c // 4, c % 4
        sl = slice(DC * g, DC * (g + 1))
        # wq/wk host layout [128, 2, NDT, 128]: [p, hp, t, c] =
        # W[t*128+p, 256g + 128hp + c] so each pair half is one contiguous
        # DMA and chains slice [:, hp, t, :].
        wq_l = (Wq[:, sl].astype(np.float16).reshape(NDT, 128, 2, 128)
                .transpose(1, 2, 0, 3))
        wk_l = (Wk[:, sl].astype(np.float16).reshape(NDT, 128, 2, 128)
                .transpose(1, 2, 0, 3))
        in_maps.append({
            "xT": np.ascontiguousarray(
                x[b].T.astype(np.float16).reshape(NDT, 128, S)
                .transpose(1, 0, 2)),
            "wq": np.ascontiguousarray(wq_l),
            "wk": np.ascontiguousarray(wk_l),
            "wv": np.ascontiguousarray(
                Wv[:, sl].astype(np.float16).reshape(NDT, 128, DC)
                .transpose(1, 0, 2)),
            "wo": np.ascontiguousarray(
                Wo[sl, :].astype(np.float16).reshape(2, 128, D)
                .transpose(1, 0, 2)),
        })
    return in_maps


def kernel(x, Wq, Wk, Wv, Wo, bo, _trace=False, _trace_cores=None):
    nc = _get_nc()
    in_maps = make_in_maps(x, Wq, Wk, Wv, Wo)
    res = bass_utils.run_bass_kernel_spmd(
        nc, in_maps, core_ids=list(range(8)), trace=_trace,
        trace_cores=_trace_cores)
    bo = np.asarray(bo, np.float32)
    out = np.empty((B, S, D), np.float32)
    for b in range(B):
        acc = res.results[4 * b]["out_p"].astype(np.float32)
        for g in range(1, 4):
            acc += res.results[4 * b + g]["out_p"].astype(np.float32)
        out[b] = acc + bo
    kernel.last_results = res
    return out


# revision 27
# speedup vs baseline: 1.0198x; 1.0168x over previous
"""Multi-head causal attention (B=2, S=2048, D=1024, H=16) on 8 TRN2 cores.

Sharding (Megatron-style): core c handles batch b = c//4, head-group
g = c%4 (4 heads, d' slice of 256). Each core computes its partial
out = ctx_g @ Wo[gslice] (no bias) in fp16; host sums the 4 partials
per batch in fp32 and adds the bias.

Device kernel dataflow (fp16 matmul operands, fp32 PSUM accumulation):
  qT/kT [d', S] and v via PE projections ->
  scores sT[k, q] per (head-pair, k-tile, q-block) (row-packed K=64
  matmul pairs) -> exp on ACT (psum->sbuf, fused 1/sqrt(hd) scale) ->
  causal triangle masking via GPSIMD affine_select in-place; fully
  masked regions are simply skipped by partial-width matmuls ->
  ctxT + softmax denominators accumulated on PE (ones columns
  interleaved in the v operand) -> 1/denom via DVE reciprocal ->
  normalize on DVE (shifted-in1 mul) -> out projection on PE.

Schedule: single j-loop interleaving both head pairs per q-block so the
ACT exp load is spread across the whole kernel instead of bunching in a
second phase (which measured ACT-bound + power-throttled). Out-proj
chunks ride at the end of each j so the output DMA streams throughout.
"""

import numpy as np

import concourse.bass as bass
import concourse.mybir as mybir
import concourse.tile as tile
from concourse import bacc, bass_utils
from concourse.hw_specs import get_activation_tables

F32 = mybir.dt.float32
F16 = mybir.dt.float16
EXP = mybir.ActivationFunctionType.Exp
LN = mybir.ActivationFunctionType.Ln

B, S, D, H, HD = 2, 2048, 1024, 16, 64
NHL = 4          # local heads per core
DC = NHL * HD    # 256 local d'
NDT = D // 128   # 8 contraction tiles for projections
KT = 128         # k tile
NKT = S // KT    # 16
QB = 512         # q block
NQB = S // QB    # 4
SCALE = 1.0 / np.sqrt(HD)

# va free-layout per k-tile m: [v0 | J | v1 | v2 | J | v3], J = ones(64)
# (matmul weight APs allow only one free dim, so the ones blocks are
# interleaved to make every head a contiguous 128-col slice).
# Head h reads 128 cols at VA_OFF[h]; even heads are [v|J] (ctx psum rows
# 0:64, denom 64:128), odd heads [J|v] (denom 0:64, ctx 64:128).
VA_W = 384
VA_OFF = [0, 64, 192, 256]


def _pin_act_table(arch):
    """Steer Bacc's activation-table chooser to the one set containing both
    exp and ln so ACT never thrashes ACT_TABLE_LOADs (1.28us each)."""
    tabs = get_activation_tables(arch)
    keep = "natural_log_exp_and_others"
    if keep not in tabs:
        return
    for name, funcs in tabs.items():
        if name != keep:
            funcs.discard(EXP)
            funcs.discard(LN)


def build_nc():
    nc = bacc.Bacc("TRN2", target_bir_lowering=False, debug=False)
    xT = nc.dram_tensor("xT", [128, NDT, S], F16, kind="ExternalInput")
    wq = nc.dram_tensor("wq", [128, 2, NDT, 128], F16, kind="ExternalInput")
    wk = nc.dram_tensor("wk", [128, 2, NDT, 128], F16, kind="ExternalInput")
    wv = nc.dram_tensor("wv", [128, NDT, DC], F16, kind="ExternalInput")
    wo = nc.dram_tensor("wo", [128, 2, D], F16, kind="ExternalInput")
    out = nc.dram_tensor("out_p", [S, D], F16, kind="ExternalOutput")

    with tile.TileContext(nc) as tc:
        with (
            tc.tile_pool(name="xp", bufs=1) as xp,
            tc.tile_pool(name="wp", bufs=1) as wp,
            tc.tile_pool(name="qk", bufs=1) as qkp,
            tc.tile_pool(name="vap", bufs=1) as vap,
            tc.tile_pool(name="cnp", bufs=1) as cnp,
            tc.tile_pool(name="et", bufs=8) as etp,
            tc.tile_pool(name="rcp", bufs=4) as rcp,
            tc.tile_pool(name="ob", bufs=4) as obp,
            tc.tile_pool(name="pp", bufs=2, space="PSUM") as pp,
            tc.tile_pool(name="sp", bufs=2, space="PSUM") as sp,
            tc.tile_pool(name="cp", bufs=1, space="PSUM") as cp,
        ):
            # ---- loads: pair-0 weight halves first so the j=0 q/k chains
            # can start as soon as x block 0 lands; x column-blocked on the
            # sync queue; wv/wo ride the vector engine's queue.
            wq_sb = wp.tile([128, 2, NDT, 128], F16, tag="wq")
            wk_sb = wp.tile([128, 2, NDT, 128], F16, tag="wk")
            wv_sb = wp.tile([128, NDT, DC], F16, tag="wv")
            wo_sb = wp.tile([128, 2, D], F16, tag="wo")
            x_sb = xp.tile([128, NDT, S], F16)
            # pair-0 q/k weights lead the scalar HWDGE queue (land ~9-11us);
            # x block 0 is split by t-halves on the sync queue so the first
            # projection chain can start accumulating after 512KiB; pair-1
            # weights ride the (slow-starting) gpsimd SWDGE queue — they
            # aren't needed until ~25us.
            # pair-0 q/k weights lead the scalar HWDGE queue; x streams in
            # column blocks on the sync queue; pair-1 weights ride the
            # gpsimd SWDGE queue; wv/wo follow on the scalar queue.
            nc.scalar.dma_start(wq_sb[:, 0], wq.ap()[:, 0])
            nc.scalar.dma_start(wk_sb[:, 0], wk.ap()[:, 0])
            for j in range(NQB):
                nc.sync.dma_start(x_sb[:, :, QB * j:QB * (j + 1)],
                                  xT.ap()[:, :, QB * j:QB * (j + 1)])
            xt_sb = [x_sb[:, t, :] for t in range(NDT)]

            # ---- constants; warmup matmuls ramp the HAM clock gate while
            # the input DMAs stream. wu needs only a fast gpsimd memset.
            wu = wp.tile([128, 128], F16, tag="wu")
            # DVE memset: the gpsimd queue is busy with SWDGE descriptor
            # generation right after its preamble, which would delay the
            # PE warmup by ~2.5us.
            nc.vector.memset(wu[:], 1.0)
            nc.gpsimd.dma_start(wq_sb[:, 1], wq.ap()[:, 1])
            nc.gpsimd.dma_start(wk_sb[:, 1], wk.ap()[:, 1])
            nc.scalar.dma_start(wv_sb[:], wv.ap())
            nc.scalar.dma_start(wo_sb[:], wo.ap())
            wups = pp.tile([128, 128], F32, tag="pp", name="wups")
            for _ in range(48):
                nc.tensor.matmul(wups[:], wu[:], wu[:], start=True, stop=True)
            tri = wp.tile([128, 128], F16, tag="tri")
            nc.gpsimd.memset(tri[:], 1.0)
            # tri[k, q] = 1 if q >= k else 0
            nc.gpsimd.affine_select(
                out=tri[:], in_=tri[:], compare_op=mybir.AluOpType.is_ge,
                fill=0.0, base=0, pattern=[[1, 128]], channel_multiplier=-1)
            # [128, NKT, 2, 192] view of the [v0|J|v1|v2|J|v3] layout: each
            # head's 128-col window lives inside one 192 block, and the
            # (v0,v2)/(v1,v3) eviction pairs and both J strips become single
            # strided APs.
            va = vap.tile([128, NKT, 2, VA_W // 2], F16)
            # only the ones-strips J; v strips are written by proj_v
            nc.gpsimd.memset(va[:, :, :, 64:128], 1.0)
            warmup = wp.tile([1, 8], F32, tag="wuact")
            nc.vector.memset(warmup[:], 1.0)
            nc.scalar.activation(warmup[:], warmup[:], EXP, scale=0.001)

            # ---- projections ----
            qT = [qkp.tile([128, S], F16, tag=f"q{hp}", name=f"q{hp}")
                  for hp in range(2)]
            kTt = [qkp.tile([128, S], F16, tag=f"k{hp}", name=f"k{hp}")
                   for hp in range(2)]
            ctxn = [cnp.tile([128, S], F16, tag=f"c{t}", name=f"c{t}")
                    for t in range(2)]

            def proj_chain(hp, dst, w_sb, j):
                ps = pp.tile([128, QB], F32, tag="pp")
                for t in range(NDT):
                    nc.tensor.matmul(
                        ps[:], w_sb[:, hp, t, :],
                        xt_sb[t][:, QB * j:QB * (j + 1)],
                        start=(t == 0), stop=(t == NDT - 1))
                nc.vector.tensor_copy(dst[:, QB * j:QB * (j + 1)], ps[:])

            def proj_v(lo, hi):
                for m in range(lo, hi):
                    ps = pp.tile([128, 2, DC // 2], F32, tag="pp")
                    for t in range(NDT):
                        nc.tensor.matmul(
                            ps[:, :, :], xt_sb[t][:, 128 * m:128 * (m + 1)],
                            wv_sb[:, t, :], start=(t == 0), stop=(t == NDT - 1))
                    # 2 strided copies: (v0,v2) -> block starts, (v1,v3) ->
                    # block offsets 128.
                    nc.vector.tensor_copy(va[:, m, :, 0:64], ps[:, :, 0:64])
                    nc.vector.tensor_copy(va[:, m, :, 128:192],
                                          ps[:, :, 64:128])

            def attention_j(hp, j, fillers=()):
                cpb = cp.tile([128, 2, QB], F32, tag="cp")  # heads 2hp, 2hp+1
                cpe, cpo = cpb[:, 0], cpb[:, 1]
                nkt_j = 4 * j + 4
                # filler PE units (proj chains / out chunks) are emitted
                # between the scores and ctx matmuls of evenly spread
                # i-tiles: within attention the ACT exp (~1.15us/tile)
                # outruns the PE (~0.86us/tile), so without filler the PE
                # starves; the first unit also covers the ctx-psum WAR on
                # the previous attention's normalize.
                nf = len(fillers)
                due = sorted(k * nkt_j // nf for k in range(nf)) if nf else []
                fi = 0
                for i in range(nkt_j):
                    c0 = max(0, 128 * (i - 4 * j))
                    spt = sp.tile([128, 2, QB], F32, tag="sp")
                    for lh in range(2):
                        nc.tensor.matmul(
                            spt[:, lh, c0:QB],
                            kTt[hp][64 * lh:64 * (lh + 1), 128 * i:128 * (i + 1)],
                            qT[hp][64 * lh:64 * (lh + 1), QB * j + c0:QB * (j + 1)],
                            start=True, stop=True)
                    while fi < nf and due[fi] == i:
                        fillers[fi]()
                        fi += 1
                    et = etp.tile([128, 2, QB], F16, tag="et")
                    nc.scalar.activation(et[:, :, c0:QB], spt[:, :, c0:QB],
                                         EXP, scale=float(SCALE))
                    if i >= 4 * j:  # diagonal: triangle mask in place
                        sl = et[:, :, c0:c0 + 128]
                        nc.gpsimd.affine_select(
                            out=sl, in_=sl,
                            compare_op=mybir.AluOpType.is_ge, fill=0.0,
                            base=0, pattern=[[0, 2], [1, 128]],
                            channel_multiplier=-1)
                    for lh, cpt in ((0, cpe), (1, cpo)):
                        off = VA_OFF[2 * hp + lh]
                        blk, o = off // 192, off % 192
                        nc.tensor.matmul(
                            cpt[:, c0:QB],
                            va[:, i, blk, o:o + 128],
                            et[:, lh, c0:QB],
                            start=(i == 0), stop=(i == nkt_j - 1))
                # normalize; even heads [v|J]: ctx rows 0:64, denom 64:128;
                # odd heads [J|v]: denom 0:64, ctx 64:128.  1/denom =
                # exp(-ln(denom)) on ACT over the whole [128, 2, QB] block
                # (one ln + one exp for both heads; ctx-row lanes produce
                # junk that is never read; DVE reciprocal measured 3.3us
                # per tile and gpsimd divide doesn't compile), then
                # shifted-in1 DVE muls (verified exact on HW).
                t1 = rcp.tile([128, 2, QB], F32, tag="t1")
                rc = rcp.tile([128, 2, QB], F32, tag="rc")
                nc.scalar.activation(t1[:], cpb[:], LN)
                nc.scalar.activation(rc[:], t1[:], EXP, scale=-1.0)
                for lh, cpt in ((0, cpe), (1, cpo)):
                    cr = slice(64 * lh, 64 * lh + 64)        # ctx rows
                    dr = slice(64 - 64 * lh, 128 - 64 * lh)  # denom rows
                    nc.vector.tensor_mul(
                        ctxn[hp][cr, QB * j:QB * (j + 1)], cpt[cr, :],
                        rc[dr, lh, :])

            def out_chunk(m, split_engines=False):
                ot = obp.tile([128, D], F16, tag="ob")
                for o in range(2):
                    ps = pp.tile([128, QB], F32, tag="pp")
                    for t in range(2):
                        nc.tensor.matmul(
                            ps[:], ctxn[t][:, 128 * m:128 * (m + 1)],
                            wo_sb[:, t, QB * o:QB * (o + 1)],
                            start=(t == 0), stop=(t == 1))
                    # evictions ride the DVE; on the last block the o=1
                    # half goes via ACT Copy so the two halves drain in
                    # parallel and the tail shortens.
                    osl = slice(QB * o, QB * (o + 1))
                    if split_engines and o == 1:
                        nc.scalar.activation(
                            ot[:, osl], ps[:],
                            mybir.ActivationFunctionType.Copy)
                    else:
                        nc.vector.tensor_copy(ot[:, osl], ps[:])
                    nc.sync.dma_start(
                        out.ap()[128 * m:128 * (m + 1), osl], ot[:, osl])

            # Interleaved schedule: per q-block j, pair-0 attention carries
            # pair-1's block-j projections plus block j-1's out-proj
            # chunks as fillers; pair-1 attention carries the NEXT block's
            # pair-0 projections and v tiles.  The final block's out
            # chunks drain with split-engine evictions.
            proj_chain(0, qT[0], wq_sb, 0)
            proj_chain(0, kTt[0], wk_sb, 0)
            proj_v(0, 4)
            for j in range(NQB):
                f0 = [lambda j=j: proj_chain(1, qT[1], wq_sb, j),
                      lambda j=j: proj_chain(1, kTt[1], wk_sb, j)]
                if j >= 1:
                    f0 += [lambda m=m: out_chunk(m)
                           for m in range(4 * (j - 1), 4 * j)]
                attention_j(0, j, f0)
                f1 = []
                if j + 1 < NQB:
                    f1 = [lambda j=j: proj_chain(0, qT[0], wq_sb, j + 1),
                          lambda j=j: proj_chain(0, kTt[0], wk_sb, j + 1)]
                    f1 += [lambda m=m: proj_v(m, m + 1)
                           for m in range(4 * (j + 1), 4 * (j + 2))]
                attention_j(1, j, f1)
            for m in range(4 * (NQB - 1), 4 * NQB):
                out_chunk(m, split_engines=True)
    _pin_act_table(nc.m.arch)
    nc.compile()
    return nc


_NC = None


def _get_nc():
    global _NC
    if _NC is None:
        _NC = build_nc()
    return _NC


def make_in_maps(x, Wq, Wk, Wv, Wo):
    x = np.asarray(x, np.float32)
    Wq, Wk, Wv, Wo = (np.asarray(w, np.float32) for w in (Wq, Wk, Wv, Wo))
    in_maps = []
    for c in range(8):
        b, g = c // 4, c % 4
        sl = slice(DC * g, DC * (g + 1))
        # wq/wk host layout [128, 2, NDT, 128]: [p, hp, t, c] =
        # W[t*128+p, 256g + 128hp + c] so each pair half is one contiguous
        # DMA and chains slice [:, hp, t, :].
        wq_l = (Wq[:, sl].astype(np.float16).reshape(NDT, 128, 2, 128)
                .transpose(1, 2, 0, 3))
        wk_l = (Wk[:, sl].astype(np.float16).reshape(NDT, 128, 2, 128)
                .transpose(1, 2, 0, 3))
        in_maps.append({
            "xT": np.ascontiguousarray(
                x[b].T.astype(np.float16).reshape(NDT, 128, S)
                .transpose(1, 0, 2)),
            "wq": np.ascontiguousarray(wq_l),
            "wk": np.ascontiguousarray(wk_l),
            "wv": np.ascontiguousarray(
                Wv[:, sl].astype(np.float16).reshape(NDT, 128, DC)
                .transpose(1, 0, 2)),
            "wo": np.ascontiguousarray(
                Wo[sl, :].astype(np.float16).reshape(2, 128, D)
                .transpose(1, 0, 2)),
        })
    return in_maps


def kernel(x, Wq, Wk, Wv, Wo, bo, _trace=False, _trace_cores=None):
    nc = _get_nc()
    in_maps = make_in_maps(x, Wq, Wk, Wv, Wo)
    res = bass_utils.run_bass_kernel_spmd(
        nc, in_maps, core_ids=list(range(8)), trace=_trace,
        trace_cores=_trace_cores)
    bo = np.asarray(bo, np.float32)
    out = np.empty((B, S, D), np.float32)
    for b in range(B):
        acc = res.results[4 * b]["out_p"].astype(np.float32)
        for g in range(1, 4):
            acc += res.results[4 * b + g]["out_p"].astype(np.float32)
        out[b] = acc + bo
    kernel.last_results = res
    return out


# revision 31
# speedup vs baseline: 1.0454x; 1.0252x over previous
"""Multi-head causal attention (B=2, S=2048, D=1024, H=16) on 8 TRN2 cores.

Sharding (Megatron-style): core c handles batch b = c//4, head-group
g = c%4 (4 heads, d' slice of 256). Each core computes its partial
out = ctx_g @ Wo[gslice] (no bias) in fp16; host sums the 4 partials
per batch in fp32 and adds the bias.

Device kernel dataflow (fp16 matmul operands, fp32 PSUM accumulation):
  qT/kT [d', S] and v via PE projections ->
  scores sT[k, q] per (head-pair, k-tile, q-block) (row-packed K=64
  matmul pairs) -> exp on ACT (psum->sbuf, fused 1/sqrt(hd) scale) ->
  causal triangle masking via GPSIMD affine_select in-place; fully
  masked regions are simply skipped by partial-width matmuls ->
  ctxT + softmax denominators accumulated on PE (ones columns
  interleaved in the v operand) -> 1/denom via DVE reciprocal ->
  normalize on DVE (shifted-in1 mul) -> out projection on PE.

Schedule: single j-loop interleaving both head pairs per q-block so the
ACT exp load is spread across the whole kernel instead of bunching in a
second phase (which measured ACT-bound + power-throttled). Out-proj
chunks ride at the end of each j so the output DMA streams throughout.
"""

import numpy as np

import concourse.bass as bass
import concourse.mybir as mybir
import concourse.tile as tile
from concourse import bacc, bass_utils
from concourse.hw_specs import get_activation_tables

F32 = mybir.dt.float32
F16 = mybir.dt.float16
EXP = mybir.ActivationFunctionType.Exp
LN = mybir.ActivationFunctionType.Ln

B, S, D, H, HD = 2, 2048, 1024, 16, 64
NHL = 4          # local heads per core
DC = NHL * HD    # 256 local d'
NDT = D // 128   # 8 contraction tiles for projections
KT = 128         # k tile
NKT = S // KT    # 16
QB = 512         # q block
NQB = S // QB    # 4
SCALE = 1.0 / np.sqrt(HD)

# va free-layout per k-tile m: [v0 | J | v1 | v2 | J | v3], J = ones(64)
# (matmul weight APs allow only one free dim, so the ones blocks are
# interleaved to make every head a contiguous 128-col slice).
# Head h reads 128 cols at VA_OFF[h]; even heads are [v|J] (ctx psum rows
# 0:64, denom 64:128), odd heads [J|v] (denom 0:64, ctx 64:128).
VA_W = 384
VA_OFF = [0, 64, 192, 256]


def _pin_act_table(arch):
    """Steer Bacc's activation-table chooser to the one set containing both
    exp and ln so ACT never thrashes ACT_TABLE_LOADs (1.28us each)."""
    tabs = get_activation_tables(arch)
    keep = "natural_log_exp_and_others"
    if keep not in tabs:
        return
    for name, funcs in tabs.items():
        if name != keep:
            funcs.discard(EXP)
            funcs.discard(LN)


def build_nc():
    nc = bacc.Bacc("TRN2", target_bir_lowering=False, debug=False)
    xT = nc.dram_tensor("xT", [128, NDT, S], F16, kind="ExternalInput")
    wq = nc.dram_tensor("wq", [128, 2, NDT, 128], F16, kind="ExternalInput")
    wk = nc.dram_tensor("wk", [128, 2, NDT, 128], F16, kind="ExternalInput")
    wv = nc.dram_tensor("wv", [128, NDT, DC], F16, kind="ExternalInput")
    wo = nc.dram_tensor("wo", [128, 2, D], F16, kind="ExternalInput")
    out = nc.dram_tensor("out_p", [S, D], F16, kind="ExternalOutput")

    with tile.TileContext(nc) as tc:
        with (
            tc.tile_pool(name="xp", bufs=1) as xp,
            tc.tile_pool(name="wp", bufs=1) as wp,
            tc.tile_pool(name="qk", bufs=1) as qkp,
            tc.tile_pool(name="vap", bufs=1) as vap,
            tc.tile_pool(name="cnp", bufs=1) as cnp,
            tc.tile_pool(name="et", bufs=8) as etp,
            tc.tile_pool(name="rcp", bufs=4) as rcp,
            tc.tile_pool(name="ob", bufs=4) as obp,
            tc.tile_pool(name="pp", bufs=2, space="PSUM") as pp,
            tc.tile_pool(name="sp", bufs=2, space="PSUM") as sp,
            tc.tile_pool(name="cp", bufs=1, space="PSUM") as cp,
        ):
            # ---- loads: pair-0 weight halves first so the j=0 q/k chains
            # can start as soon as x block 0 lands; x column-blocked on the
            # sync queue; wv/wo ride the vector engine's queue.
            wq_sb = wp.tile([128, 2, NDT, 128], F16, tag="wq")
            wk_sb = wp.tile([128, 2, NDT, 128], F16, tag="wk")
            wv_sb = wp.tile([128, NDT, DC], F16, tag="wv")
            wo_sb = wp.tile([128, 2, D], F16, tag="wo")
            x_sb = xp.tile([128, NDT, S], F16)
            # pair-0 q/k weights lead the scalar HWDGE queue (land ~9-11us);
            # x block 0 is split by t-halves on the sync queue so the first
            # projection chain can start accumulating after 512KiB; pair-1
            # weights ride the (slow-starting) gpsimd SWDGE queue — they
            # aren't needed until ~25us.
            # pair-0 q/k weights lead the scalar HWDGE queue; x block 0
            # (t-halves) + block 1 get the sync queue to themselves (a
            # queue interleaves its pending DMAs, so fewer rivals = block 0
            # lands sooner); x2/x3 follow the weights on the scalar queue;
            # pair-1 weights ride the gpsimd SWDGE queue.
            nc.scalar.dma_start(wq_sb[:, 0], wq.ap()[:, 0])
            nc.scalar.dma_start(wk_sb[:, 0], wk.ap()[:, 0])
            nc.sync.dma_start(x_sb[:, 0:4, 0:QB], xT.ap()[:, 0:4, 0:QB])
            nc.sync.dma_start(x_sb[:, 4:8, 0:QB], xT.ap()[:, 4:8, 0:QB])
            if NQB > 1:
                nc.sync.dma_start(x_sb[:, :, QB:2 * QB],
                                  xT.ap()[:, :, QB:2 * QB])
            xt_sb = [x_sb[:, t, :] for t in range(NDT)]

            # ---- constants; warmup matmuls ramp the HAM clock gate while
            # the input DMAs stream. wu needs only a fast gpsimd memset.
            wu = wp.tile([128, 128], F16, tag="wu")
            # DVE memset: the gpsimd queue is busy with SWDGE descriptor
            # generation right after its preamble, which would delay the
            # PE warmup by ~2.5us.
            nc.vector.memset(wu[:], 1.0)
            nc.gpsimd.dma_start(wq_sb[:, 1], wq.ap()[:, 1])
            nc.gpsimd.dma_start(wk_sb[:, 1], wk.ap()[:, 1])
            nc.scalar.dma_start(wv_sb[:], wv.ap())
            nc.scalar.dma_start(wo_sb[:], wo.ap())
            for j in range(2, NQB):
                nc.scalar.dma_start(x_sb[:, :, QB * j:QB * (j + 1)],
                                    xT.ap()[:, :, QB * j:QB * (j + 1)])
            wups = pp.tile([128, 128], F32, tag="pp", name="wups")
            for _ in range(48):
                nc.tensor.matmul(wups[:], wu[:], wu[:], start=True, stop=True)
            tri = wp.tile([128, 128], F16, tag="tri")
            nc.gpsimd.memset(tri[:], 1.0)
            # tri[k, q] = 1 if q >= k else 0
            nc.gpsimd.affine_select(
                out=tri[:], in_=tri[:], compare_op=mybir.AluOpType.is_ge,
                fill=0.0, base=0, pattern=[[1, 128]], channel_multiplier=-1)
            # [128, NKT, 2, 192] view of the [v0|J|v1|v2|J|v3] layout: each
            # head's 128-col window lives inside one 192 block, and the
            # (v0,v2)/(v1,v3) eviction pairs and both J strips become single
            # strided APs.
            va = vap.tile([128, NKT, 2, VA_W // 2], F16)
            # only the ones-strips J; v strips are written by proj_v
            nc.gpsimd.memset(va[:, :, :, 64:128], 1.0)
            warmup = wp.tile([1, 8], F32, tag="wuact")
            nc.vector.memset(warmup[:], 1.0)
            nc.scalar.activation(warmup[:], warmup[:], EXP, scale=0.001)

            # ---- projections ----
            qT = [qkp.tile([128, S], F16, tag=f"q{hp}", name=f"q{hp}")
                  for hp in range(2)]
            kTt = [qkp.tile([128, S], F16, tag=f"k{hp}", name=f"k{hp}")
                   for hp in range(2)]
            ctxn = [cnp.tile([128, S], F16, tag=f"c{t}", name=f"c{t}")
                    for t in range(2)]

            def proj_chain(hp, dst, w_sb, j):
                ps = pp.tile([128, QB], F32, tag="pp")
                for t in range(NDT):
                    nc.tensor.matmul(
                        ps[:], w_sb[:, hp, t, :],
                        xt_sb[t][:, QB * j:QB * (j + 1)],
                        start=(t == 0), stop=(t == NDT - 1))
                nc.vector.tensor_copy(dst[:, QB * j:QB * (j + 1)], ps[:])

            def proj_v(lo, hi):
                for m in range(lo, hi):
                    ps = pp.tile([128, 2, DC // 2], F32, tag="pp")
                    for t in range(NDT):
                        nc.tensor.matmul(
                            ps[:, :, :], xt_sb[t][:, 128 * m:128 * (m + 1)],
                            wv_sb[:, t, :], start=(t == 0), stop=(t == NDT - 1))
                    # 2 strided copies: (v0,v2) -> block starts, (v1,v3) ->
                    # block offsets 128.
                    nc.vector.tensor_copy(va[:, m, :, 0:64], ps[:, :, 0:64])
                    nc.vector.tensor_copy(va[:, m, :, 128:192],
                                          ps[:, :, 64:128])

            def attention_j(hp, j, fillers=()):
                cpb = cp.tile([128, 2, QB], F32, tag="cp")  # heads 2hp, 2hp+1
                cpe, cpo = cpb[:, 0], cpb[:, 1]
                nkt_j = 4 * j + 4
                # filler PE units (proj chains / out chunks) are emitted
                # between the scores and ctx matmuls of evenly spread
                # i-tiles: within attention the ACT exp (~1.15us/tile)
                # outruns the PE (~0.86us/tile), so without filler the PE
                # starves; the first unit also covers the ctx-psum WAR on
                # the previous attention's normalize.
                nf = len(fillers)
                due = sorted(k * nkt_j // nf for k in range(nf)) if nf else []
                fi = 0
                for i in range(nkt_j):
                    c0 = max(0, 128 * (i - 4 * j))
                    spt = sp.tile([128, 2, QB], F32, tag="sp")
                    for lh in range(2):
                        nc.tensor.matmul(
                            spt[:, lh, c0:QB],
                            kTt[hp][64 * lh:64 * (lh + 1), 128 * i:128 * (i + 1)],
                            qT[hp][64 * lh:64 * (lh + 1), QB * j + c0:QB * (j + 1)],
                            start=True, stop=True)
                    while fi < nf and due[fi] == i:
                        fillers[fi]()
                        fi += 1
                    et = etp.tile([128, 2, QB], F16, tag="et")
                    nc.scalar.activation(et[:, :, c0:QB], spt[:, :, c0:QB],
                                         EXP, scale=float(SCALE))
                    if i >= 4 * j:  # diagonal: triangle mask in place
                        sl = et[:, :, c0:c0 + 128]
                        nc.gpsimd.affine_select(
                            out=sl, in_=sl,
                            compare_op=mybir.AluOpType.is_ge, fill=0.0,
                            base=0, pattern=[[0, 2], [1, 128]],
                            channel_multiplier=-1)
                    for lh, cpt in ((0, cpe), (1, cpo)):
                        off = VA_OFF[2 * hp + lh]
                        blk, o = off // 192, off % 192
                        nc.tensor.matmul(
                            cpt[:, c0:QB],
                            va[:, i, blk, o:o + 128],
                            et[:, lh, c0:QB],
                            start=(i == 0), stop=(i == nkt_j - 1))
                # normalize; even heads [v|J]: ctx rows 0:64, denom 64:128;
                # odd heads [J|v]: denom 0:64, ctx 64:128.  1/denom =
                # exp(-ln(denom)) on ACT over the whole [128, 2, QB] block
                # (one ln + one exp for both heads; ctx-row lanes produce
                # junk that is never read; DVE reciprocal measured 3.3us
                # per tile and gpsimd divide doesn't compile), then
                # shifted-in1 DVE muls (verified exact on HW).
                t1 = rcp.tile([128, 2, QB], F32, tag="t1")
                rc = rcp.tile([128, 2, QB], F32, tag="rc")
                nc.scalar.activation(t1[:], cpb[:], LN)
                nc.scalar.activation(rc[:], t1[:], EXP, scale=-1.0)
                for lh, cpt in ((0, cpe), (1, cpo)):
                    cr = slice(64 * lh, 64 * lh + 64)        # ctx rows
                    dr = slice(64 - 64 * lh, 128 - 64 * lh)  # denom rows
                    nc.vector.tensor_mul(
                        ctxn[hp][cr, QB * j:QB * (j + 1)], cpt[cr, :],
                        rc[dr, lh, :])

            def out_chunk(m, split_engines=False):
                ot = obp.tile([128, D], F16, tag="ob")
                for o in range(2):
                    ps = pp.tile([128, QB], F32, tag="pp")
                    for t in range(2):
                        nc.tensor.matmul(
                            ps[:], ctxn[t][:, 128 * m:128 * (m + 1)],
                            wo_sb[:, t, QB * o:QB * (o + 1)],
                            start=(t == 0), stop=(t == 1))
                    # evictions ride the DVE; on the last block the o=1
                    # half goes via ACT Copy so the two halves drain in
                    # parallel and the tail shortens.
                    osl = slice(QB * o, QB * (o + 1))
                    if split_engines and o == 1:
                        nc.scalar.activation(
                            ot[:, osl], ps[:],
                            mybir.ActivationFunctionType.Copy)
                    else:
                        nc.vector.tensor_copy(ot[:, osl], ps[:])
                    nc.sync.dma_start(
                        out.ap()[128 * m:128 * (m + 1), osl], ot[:, osl])

            def out_epilogue():
                """Final block's out chunks: all pair-0 accumulation halves
                are emitted first into idle psum (pp + sp + cp pools, free
                after the last attention) so the PE crunches them during
                the final normalize; then the pair-1 halves + evictions
                stream out on both ACT and DVE."""
                ms = list(range(4 * (NQB - 1), 4 * NQB))
                tiles = {}
                half = []
                for k, m in enumerate(ms):
                    if k == 0:
                        a = pp.tile([128, QB], F32, tag="pp")
                        b = pp.tile([128, QB], F32, tag="pp")
                    elif k == 3:
                        cb = cp.tile([128, 2, QB], F32, tag="cp")
                        a, b = cb[:, 0], cb[:, 1]
                    else:
                        sb_ = sp.tile([128, 2, QB], F32, tag="sp")
                        a, b = sb_[:, 0], sb_[:, 1]
                    tiles[m] = (a, b)
                for m in ms:
                    for o in range(2):
                        nc.tensor.matmul(
                            tiles[m][o], ctxn[0][:, 128 * m:128 * (m + 1)],
                            wo_sb[:, 0, QB * o:QB * (o + 1)],
                            start=True, stop=False)
                for m in ms:
                    ot = obp.tile([128, D], F16, tag="ob")
                    for o in range(2):
                        nc.tensor.matmul(
                            tiles[m][o], ctxn[1][:, 128 * m:128 * (m + 1)],
                            wo_sb[:, 1, QB * o:QB * (o + 1)],
                            start=False, stop=True)
                        osl = slice(QB * o, QB * (o + 1))
                        if o == 1:
                            nc.scalar.activation(
                                ot[:, osl], tiles[m][o],
                                mybir.ActivationFunctionType.Copy)
                        else:
                            nc.vector.tensor_copy(ot[:, osl], tiles[m][o])
                        nc.sync.dma_start(
                            out.ap()[128 * m:128 * (m + 1), osl], ot[:, osl])

            # Interleaved schedule: per q-block j, pair-0 attention carries
            # pair-1's block-j projections as fillers; pair-1 attention
            # carries the NEXT block's pair-0 projections and v tiles.
            # Completed blocks' out-proj chunks are DEFERRED and spent as
            # fillers in the LAST block's two attentions — those are the
            # longest and have no projection work left, so without the
            # deferred chunks the PE starves behind ACT exp there.
            proj_chain(0, qT[0], wq_sb, 0)
            proj_chain(0, kTt[0], wk_sb, 0)
            proj_v(0, 4)
            pending = []      # out-chunk ms whose ctxn blocks are complete
            for j in range(NQB):
                f0 = [lambda j=j: proj_chain(1, qT[1], wq_sb, j),
                      lambda j=j: proj_chain(1, kTt[1], wk_sb, j)]
                if j == NQB - 1:
                    f0 += [lambda m=m: out_chunk(m) for m in pending[:4]]
                    pending = pending[4:]
                attention_j(0, j, f0)
                f1 = []
                if j + 1 < NQB:
                    f1 = [lambda j=j: proj_chain(0, qT[0], wq_sb, j + 1),
                          lambda j=j: proj_chain(0, kTt[0], wk_sb, j + 1)]
                    f1 += [lambda m=m: proj_v(m, m + 1)
                           for m in range(4 * (j + 1), 4 * (j + 2))]
                if j == 1 or j == NQB - 1:
                    f1 += [lambda m=m: out_chunk(m) for m in pending[:4]]
                    pending = pending[4:]
                attention_j(1, j, f1)
                if j < NQB - 1:
                    pending += list(range(4 * j, 4 * j + 4))
            assert not pending, pending
            out_epilogue()
    _pin_act_table(nc.m.arch)
    nc.compile()
    return nc


_NC = None


def _get_nc():
    global _NC
    if _NC is None:
        _NC = build_nc()
    return _NC


def make_in_maps(x, Wq, Wk, Wv, Wo):
    x = np.asarray(x, np.float32)
    Wq, Wk, Wv, Wo = (np.asarray(w, np.float32) for w in (Wq, Wk, Wv, Wo))
    in_maps = []
    for c in range(8):
        b, g = c // 4, c % 4
        sl = slice(DC * g, DC * (g + 1))
        # wq/wk host layout [128, 2, NDT, 128]: [p, hp, t, c] =
        # W[t*128+p, 256g + 128hp + c] so each pair half is one contiguous
        # DMA and chains slice [:, hp, t, :].
        wq_l = (Wq[:, sl].astype(np.float16).reshape(NDT, 128, 2, 128)
                .transpose(1, 2, 0, 3))
        wk_l = (Wk[:, sl].astype(np.float16).reshape(NDT, 128, 2, 128)
                .transpose(1, 2, 0, 3))
        in_maps.append({
            "xT": np.ascontiguousarray(
                x[b].T.astype(np.float16).reshape(NDT, 128, S)
                .transpose(1, 0, 2)),
            "wq": np.ascontiguousarray(wq_l),
            "wk": np.ascontiguousarray(wk_l),
            "wv": np.ascontiguousarray(
                Wv[:, sl].astype(np.float16).reshape(NDT, 128, DC)
                .transpose(1, 0, 2)),
            "wo": np.ascontiguousarray(
                Wo[sl, :].astype(np.float16).reshape(2, 128, D)
                .transpose(1, 0, 2)),
        })
    return in_maps


def kernel(x, Wq, Wk, Wv, Wo, bo, _trace=False, _trace_cores=None):
    nc = _get_nc()
    in_maps = make_in_maps(x, Wq, Wk, Wv, Wo)
    res = bass_utils.run_bass_kernel_spmd(
        nc, in_maps, core_ids=list(range(8)), trace=_trace,
        trace_cores=_trace_cores)
    bo = np.asarray(bo, np.float32)
    out = np.empty((B, S, D), np.float32)
    for b in range(B):
        acc = res.results[4 * b]["out_p"].astype(np.float32)
        for g in range(1, 4):
            acc += res.results[4 * b + g]["out_p"].astype(np.float32)
        out[b] = acc + bo
    kernel.last_results = res
    return out


# revision 34
# speedup vs baseline: 1.0659x; 1.0195x over previous
"""Multi-head causal attention (B=2, S=2048, D=1024, H=16) on 8 TRN2 cores.

Sharding (Megatron-style): core c handles batch b = c//4, head-group
g = c%4 (4 heads, d' slice of 256). Each core computes its partial
out = ctx_g @ Wo[gslice] (no bias) in fp16; host sums the 4 partials
per batch in fp32 and adds the bias.

Device kernel dataflow (fp16 matmul operands, fp32 PSUM accumulation):
  qT/kT [d', S] and v via PE projections ->
  scores sT[k, q] per (head-pair, k-tile, q-block) (row-packed K=64
  matmul pairs) -> exp on ACT (psum->sbuf, fused 1/sqrt(hd) scale) ->
  causal triangle masking via GPSIMD affine_select in-place; fully
  masked regions are simply skipped by partial-width matmuls ->
  ctxT + softmax denominators accumulated on PE (ones columns
  interleaved in the v operand) -> 1/denom via DVE reciprocal ->
  normalize on DVE (shifted-in1 mul) -> out projection on PE.

Schedule: single j-loop interleaving both head pairs per q-block so the
ACT exp load is spread across the whole kernel instead of bunching in a
second phase (which measured ACT-bound + power-throttled). Out-proj
chunks ride at the end of each j so the output DMA streams throughout.
"""

import numpy as np

import concourse.bass as bass
import concourse.mybir as mybir
import concourse.tile as tile
from concourse import bacc, bass_utils
from concourse.hw_specs import get_activation_tables

F32 = mybir.dt.float32
F16 = mybir.dt.float16
EXP = mybir.ActivationFunctionType.Exp
LN = mybir.ActivationFunctionType.Ln

B, S, D, H, HD = 2, 2048, 1024, 16, 64
NHL = 4          # local heads per core
DC = NHL * HD    # 256 local d'
NDT = D // 128   # 8 contraction tiles for projections
KT = 128         # k tile
NKT = S // KT    # 16
QB = 512         # q block
NQB = S // QB    # 4
SCALE = 1.0 / np.sqrt(HD)

# va free-layout per k-tile m: [v0 | J | v1 | v2 | J | v3], J = ones(64)
# (matmul weight APs allow only one free dim, so the ones blocks are
# interleaved to make every head a contiguous 128-col slice).
# Head h reads 128 cols at VA_OFF[h]; even heads are [v|J] (ctx psum rows
# 0:64, denom 64:128), odd heads [J|v] (denom 0:64, ctx 64:128).
VA_W = 384
VA_OFF = [0, 64, 192, 256]


def _pin_act_table(arch):
    """Steer Bacc's activation-table chooser to the one set containing both
    exp and ln so ACT never thrashes ACT_TABLE_LOADs (1.28us each)."""
    tabs = get_activation_tables(arch)
    keep = "natural_log_exp_and_others"
    if keep not in tabs:
        return
    for name, funcs in tabs.items():
        if name != keep:
            funcs.discard(EXP)
            funcs.discard(LN)


def build_nc():
    nc = bacc.Bacc("TRN2", target_bir_lowering=False, debug=False)
    xT = nc.dram_tensor("xT", [128, NDT, S], F16, kind="ExternalInput")
    wq = nc.dram_tensor("wq", [128, 2, NDT, 128], F16, kind="ExternalInput")
    wk = nc.dram_tensor("wk", [128, 2, NDT, 128], F16, kind="ExternalInput")
    wv = nc.dram_tensor("wv", [128, NDT, DC], F16, kind="ExternalInput")
    wo = nc.dram_tensor("wo", [128, 2, D], F16, kind="ExternalInput")
    out = nc.dram_tensor("out_p", [S, D], F16, kind="ExternalOutput")

    with tile.TileContext(nc) as tc:
        with (
            tc.tile_pool(name="xp", bufs=1) as xp,
            tc.tile_pool(name="wp", bufs=1) as wp,
            tc.tile_pool(name="qk", bufs=1) as qkp,
            tc.tile_pool(name="vap", bufs=1) as vap,
            tc.tile_pool(name="cnp", bufs=1) as cnp,
            tc.tile_pool(name="et", bufs=8) as etp,
            tc.tile_pool(name="rcp", bufs=4) as rcp,
            tc.tile_pool(name="ob", bufs=4) as obp,
            tc.tile_pool(name="pp", bufs=2, space="PSUM") as pp,
            tc.tile_pool(name="sp", bufs=2, space="PSUM") as sp,
            tc.tile_pool(name="cp", bufs=1, space="PSUM") as cp,
        ):
            # ---- loads: pair-0 weight halves first so the j=0 q/k chains
            # can start as soon as x block 0 lands; x column-blocked on the
            # sync queue; wv/wo ride the vector engine's queue.
            wq_sb = wp.tile([128, 2, NDT, 128], F16, tag="wq")
            wk_sb = wp.tile([128, 2, NDT, 128], F16, tag="wk")
            wv_sb = wp.tile([128, NDT, DC], F16, tag="wv")
            wo_sb = wp.tile([128, 2, D], F16, tag="wo")
            x_sb = xp.tile([128, NDT, S], F16)
            # pair-0 q/k weights lead the scalar HWDGE queue (land ~9-11us);
            # x block 0 is split by t-halves on the sync queue so the first
            # projection chain can start accumulating after 512KiB; pair-1
            # weights ride the (slow-starting) gpsimd SWDGE queue — they
            # aren't needed until ~25us.
            # pair-0 q/k weights lead the scalar HWDGE queue; x block 0
            # (t-halves) + block 1 get the sync queue to themselves (a
            # queue interleaves its pending DMAs, so fewer rivals = block 0
            # lands sooner); x2/x3 follow the weights on the scalar queue;
            # pair-1 weights ride the gpsimd SWDGE queue.
            nc.scalar.dma_start(wq_sb[:, 0], wq.ap()[:, 0])
            nc.scalar.dma_start(wk_sb[:, 0], wk.ap()[:, 0])
            nc.sync.dma_start(x_sb[:, 0:4, 0:QB], xT.ap()[:, 0:4, 0:QB])
            nc.sync.dma_start(x_sb[:, 4:8, 0:QB], xT.ap()[:, 4:8, 0:QB])
            xt_sb = [x_sb[:, t, :] for t in range(NDT)]

            # ---- constants; warmup matmuls ramp the HAM clock gate while
            # the input DMAs stream. wu needs only a fast gpsimd memset.
            wu = wp.tile([128, 128], F16, tag="wu")
            # DVE memset: the gpsimd queue is busy with SWDGE descriptor
            # generation right after its preamble, which would delay the
            # PE warmup by ~2.5us.
            nc.vector.memset(wu[:], 1.0)
            nc.gpsimd.dma_start(wq_sb[:, 1], wq.ap()[:, 1])
            nc.gpsimd.dma_start(wk_sb[:, 1], wk.ap()[:, 1])
            nc.scalar.dma_start(wv_sb[:], wv.ap())
            if NQB > 1:
                nc.scalar.dma_start(x_sb[:, :, QB:2 * QB],
                                    xT.ap()[:, :, QB:2 * QB])
            nc.scalar.dma_start(wo_sb[:], wo.ap())
            for j in range(2, NQB):
                nc.scalar.dma_start(x_sb[:, :, QB * j:QB * (j + 1)],
                                    xT.ap()[:, :, QB * j:QB * (j + 1)])
            wups = pp.tile([128, 128], F32, tag="pp", name="wups")
            for _ in range(48):
                nc.tensor.matmul(wups[:], wu[:], wu[:], start=True, stop=True)
            tri = wp.tile([128, 128], F16, tag="tri")
            nc.gpsimd.memset(tri[:], 1.0)
            # tri[k, q] = 1 if q >= k else 0
            nc.gpsimd.affine_select(
                out=tri[:], in_=tri[:], compare_op=mybir.AluOpType.is_ge,
                fill=0.0, base=0, pattern=[[1, 128]], channel_multiplier=-1)
            # [128, NKT, 2, 192] view of the [v0|J|v1|v2|J|v3] layout: each
            # head's 128-col window lives inside one 192 block, and the
            # (v0,v2)/(v1,v3) eviction pairs and both J strips become single
            # strided APs.
            va = vap.tile([128, NKT, 2, VA_W // 2], F16)
            # only the ones-strips J; v strips are written by proj_v
            nc.gpsimd.memset(va[:, :, :, 64:128], 1.0)
            warmup = wp.tile([1, 8], F32, tag="wuact")
            nc.vector.memset(warmup[:], 1.0)
            nc.scalar.activation(warmup[:], warmup[:], EXP, scale=0.001)

            # ---- projections ----
            qT = [qkp.tile([128, S], F16, tag=f"q{hp}", name=f"q{hp}")
                  for hp in range(2)]
            kTt = [qkp.tile([128, S], F16, tag=f"k{hp}", name=f"k{hp}")
                   for hp in range(2)]
            ctxn = [cnp.tile([128, S], F16, tag=f"c{t}", name=f"c{t}")
                    for t in range(2)]

            def proj_chain(hp, dst, w_sb, j):
                ps = pp.tile([128, QB], F32, tag="pp")
                for t in range(NDT):
                    nc.tensor.matmul(
                        ps[:], w_sb[:, hp, t, :],
                        xt_sb[t][:, QB * j:QB * (j + 1)],
                        start=(t == 0), stop=(t == NDT - 1))
                nc.vector.tensor_copy(dst[:, QB * j:QB * (j + 1)], ps[:])

            def proj_v(lo, hi):
                for m in range(lo, hi):
                    ps = pp.tile([128, 2, DC // 2], F32, tag="pp")
                    for t in range(NDT):
                        nc.tensor.matmul(
                            ps[:, :, :], xt_sb[t][:, 128 * m:128 * (m + 1)],
                            wv_sb[:, t, :], start=(t == 0), stop=(t == NDT - 1))
                    # 2 strided copies: (v0,v2) -> block starts, (v1,v3) ->
                    # block offsets 128.
                    nc.vector.tensor_copy(va[:, m, :, 0:64], ps[:, :, 0:64])
                    nc.vector.tensor_copy(va[:, m, :, 128:192],
                                          ps[:, :, 64:128])

            def attention_j(hp, j, fillers=()):
                cpb = cp.tile([128, 2, QB], F32, tag="cp")  # heads 2hp, 2hp+1
                cpe, cpo = cpb[:, 0], cpb[:, 1]
                nkt_j = 4 * j + 4
                # filler PE units (proj chains / out chunks) are emitted
                # between the scores and ctx matmuls of evenly spread
                # i-tiles: within attention the ACT exp (~1.15us/tile)
                # outruns the PE (~0.86us/tile), so without filler the PE
                # starves; the first unit also covers the ctx-psum WAR on
                # the previous attention's normalize.
                nf = len(fillers)
                due = sorted(k * nkt_j // nf for k in range(nf)) if nf else []
                fi = 0
                for i in range(nkt_j):
                    c0 = max(0, 128 * (i - 4 * j))
                    spt = sp.tile([128, 2, QB], F32, tag="sp")
                    for lh in range(2):
                        nc.tensor.matmul(
                            spt[:, lh, c0:QB],
                            kTt[hp][64 * lh:64 * (lh + 1), 128 * i:128 * (i + 1)],
                            qT[hp][64 * lh:64 * (lh + 1), QB * j + c0:QB * (j + 1)],
                            start=True, stop=True)
                    while fi < nf and due[fi] == i:
                        fillers[fi]()
                        fi += 1
                    et = etp.tile([128, 2, QB], F16, tag="et")
                    nc.scalar.activation(et[:, :, c0:QB], spt[:, :, c0:QB],
                                         EXP, scale=float(SCALE))
                    if i >= 4 * j:  # diagonal: triangle mask in place
                        sl = et[:, :, c0:c0 + 128]
                        nc.gpsimd.affine_select(
                            out=sl, in_=sl,
                            compare_op=mybir.AluOpType.is_ge, fill=0.0,
                            base=0, pattern=[[0, 2], [1, 128]],
                            channel_multiplier=-1)
                    for lh, cpt in ((0, cpe), (1, cpo)):
                        off = VA_OFF[2 * hp + lh]
                        blk, o = off // 192, off % 192
                        nc.tensor.matmul(
                            cpt[:, c0:QB],
                            va[:, i, blk, o:o + 128],
                            et[:, lh, c0:QB],
                            start=(i == 0), stop=(i == nkt_j - 1))
                # normalize; even heads [v|J]: ctx rows 0:64, denom 64:128;
                # odd heads [J|v]: denom 0:64, ctx 64:128.  1/denom =
                # exp(-ln(denom)) on ACT over the whole [128, 2, QB] block
                # (one ln + one exp for both heads; ctx-row lanes produce
                # junk that is never read; DVE reciprocal measured 3.3us
                # per tile and gpsimd divide doesn't compile), then
                # shifted-in1 DVE muls (verified exact on HW).
                t1 = rcp.tile([128, 2, QB], F32, tag="t1")
                rc = rcp.tile([128, 2, QB], F32, tag="rc")
                nc.scalar.activation(t1[:], cpb[:], LN)
                nc.scalar.activation(rc[:], t1[:], EXP, scale=-1.0)
                for lh, cpt in ((0, cpe), (1, cpo)):
                    cr = slice(64 * lh, 64 * lh + 64)        # ctx rows
                    dr = slice(64 - 64 * lh, 128 - 64 * lh)  # denom rows
                    nc.vector.tensor_mul(
                        ctxn[hp][cr, QB * j:QB * (j + 1)], cpt[cr, :],
                        rc[dr, lh, :])

            def out_chunk(m, split_engines=False):
                ot = obp.tile([128, D], F16, tag="ob")
                for o in range(2):
                    ps = pp.tile([128, QB], F32, tag="pp")
                    for t in range(2):
                        nc.tensor.matmul(
                            ps[:], ctxn[t][:, 128 * m:128 * (m + 1)],
                            wo_sb[:, t, QB * o:QB * (o + 1)],
                            start=(t == 0), stop=(t == 1))
                    # evictions ride the DVE; on the last block the o=1
                    # half goes via ACT Copy so the two halves drain in
                    # parallel and the tail shortens.
                    osl = slice(QB * o, QB * (o + 1))
                    if split_engines and o == 1:
                        nc.scalar.activation(
                            ot[:, osl], ps[:],
                            mybir.ActivationFunctionType.Copy)
                    else:
                        nc.vector.tensor_copy(ot[:, osl], ps[:])
                    nc.sync.dma_start(
                        out.ap()[128 * m:128 * (m + 1), osl], ot[:, osl])

            def out_epilogue():
                """Final block's out chunks: all pair-0 accumulation halves
                are emitted first into idle psum (pp + sp + cp pools, free
                after the last attention) so the PE crunches them during
                the final normalize; then the pair-1 halves + evictions
                stream out on both ACT and DVE."""
                ms = list(range(4 * (NQB - 1), 4 * NQB))
                tiles = {}
                half = []
                for k, m in enumerate(ms):
                    if k == 0:
                        a = pp.tile([128, QB], F32, tag="pp")
                        b = pp.tile([128, QB], F32, tag="pp")
                    elif k == 3:
                        cb = cp.tile([128, 2, QB], F32, tag="cp")
                        a, b = cb[:, 0], cb[:, 1]
                    else:
                        sb_ = sp.tile([128, 2, QB], F32, tag="sp")
                        a, b = sb_[:, 0], sb_[:, 1]
                    tiles[m] = (a, b)
                for m in ms:
                    for o in range(2):
                        nc.tensor.matmul(
                            tiles[m][o], ctxn[0][:, 128 * m:128 * (m + 1)],
                            wo_sb[:, 0, QB * o:QB * (o + 1)],
                            start=True, stop=False)
                for k, m in enumerate(ms):
                    ot = obp.tile([128, D], F16, tag="ob")
                    for o in range(2):
                        nc.tensor.matmul(
                            tiles[m][o], ctxn[1][:, 128 * m:128 * (m + 1)],
                            wo_sb[:, 1, QB * o:QB * (o + 1)],
                            start=False, stop=True)
                        osl = slice(QB * o, QB * (o + 1))
                        if o == 1:
                            nc.scalar.activation(
                                ot[:, osl], tiles[m][o],
                                mybir.ActivationFunctionType.Copy)
                        else:
                            nc.vector.tensor_copy(ot[:, osl], tiles[m][o])
                    # one whole-chunk DMA per m, alternating queues — the
                    # tail otherwise serializes ~8x600ns of DMA issue on
                    # the sync engine.
                    eng = nc.sync if k % 2 == 0 else nc.scalar
                    eng.dma_start(out.ap()[128 * m:128 * (m + 1), :], ot[:])

            # Interleaved schedule: per q-block j, pair-0 attention carries
            # pair-1's block-j projections as fillers; pair-1 attention
            # carries the NEXT block's pair-0 projections and v tiles.
            # Completed blocks' out-proj chunks are DEFERRED and spent as
            # fillers in the LAST block's two attentions — those are the
            # longest and have no projection work left, so without the
            # deferred chunks the PE starves behind ACT exp there.
            proj_chain(0, qT[0], wq_sb, 0)
            proj_chain(0, kTt[0], wk_sb, 0)
            proj_v(0, 4)
            pending = []      # out-chunk ms whose ctxn blocks are complete
            for j in range(NQB):
                f0 = [lambda j=j: proj_chain(1, qT[1], wq_sb, j),
                      lambda j=j: proj_chain(1, kTt[1], wk_sb, j)]
                if j == NQB - 1:
                    f0 += [lambda m=m: out_chunk(m) for m in pending[:4]]
                    pending = pending[4:]
                attention_j(0, j, f0)
                f1 = []
                if j + 1 < NQB:
                    f1 = [lambda j=j: proj_chain(0, qT[0], wq_sb, j + 1),
                          lambda j=j: proj_chain(0, kTt[0], wk_sb, j + 1)]
                    f1 += [lambda m=m: proj_v(m, m + 1)
                           for m in range(4 * (j + 1), 4 * (j + 2))]
                if j == 1 or j == NQB - 1:
                    f1 += [lambda m=m: out_chunk(m) for m in pending[:4]]
                    pending = pending[4:]
                attention_j(1, j, f1)
                if j < NQB - 1:
                    pending += list(range(4 * j, 4 * j + 4))
            assert not pending, pending
            out_epilogue()
    _pin_act_table(nc.m.arch)
    nc.compile()
    return nc


_NC = None


def _get_nc():
    global _NC
    if _NC is None:
        _NC = build_nc()
    return _NC


def make_in_maps(x, Wq, Wk, Wv, Wo):
    x = np.asarray(x, np.float32)
    Wq, Wk, Wv, Wo = (np.asarray(w, np.float32) for w in (Wq, Wk, Wv, Wo))
    in_maps = []
    for c in range(8):
        b, g = c // 4, c % 4
        sl = slice(DC * g, DC * (g + 1))
        # wq/wk host layout [128, 2, NDT, 128]: [p, hp, t, c] =
        # W[t*128+p, 256g + 128hp + c] so each pair half is one contiguous
        # DMA and chains slice [:, hp, t, :].
        wq_l = (Wq[:, sl].astype(np.float16).reshape(NDT, 128, 2, 128)
                .transpose(1, 2, 0, 3))
        wk_l = (Wk[:, sl].astype(np.float16).reshape(NDT, 128, 2, 128)
                .transpose(1, 2, 0, 3))
        in_maps.append({
            "xT": np.ascontiguousarray(
                x[b].T.astype(np.float16).reshape(NDT, 128, S)
                .transpose(1, 0, 2)),
            "wq": np.ascontiguousarray(wq_l),
            "wk": np.ascontiguousarray(wk_l),
            "wv": np.ascontiguousarray(
                Wv[:, sl].astype(np.float16).reshape(NDT, 128, DC)
                .transpose(1, 0, 2)),
            "wo": np.ascontiguousarray(
                Wo[sl, :].astype(np.float16).reshape(2, 128, D)
                .transpose(1, 0, 2)),
        })
    return in_maps


def kernel(x, Wq, Wk, Wv, Wo, bo, _trace=False, _trace_cores=None):
    nc = _get_nc()
    in_maps = make_in_maps(x, Wq, Wk, Wv, Wo)
    res = bass_utils.run_bass_kernel_spmd(
        nc, in_maps, core_ids=list(range(8)), trace=_trace,
        trace_cores=_trace_cores)
    bo = np.asarray(bo, np.float32)
    out = np.empty((B, S, D), np.float32)
    for b in range(B):
        acc = res.results[4 * b]["out_p"].astype(np.float32)
        for g in range(1, 4):
            acc += res.results[4 * b + g]["out_p"].astype(np.float32)
        out[b] = acc + bo
    kernel.last_results = res
    return out
